# revision 1
# baseline (speedup 1.0000x reference)
"""Trainium2 Bass kernel for 2-layer LSTM (H=32, in=1) + final-step FC.

Problem: x [4096, 1024, 1] -> 2x LSTM(H=32) -> h2[:, -1, :] @ Wfc.T + bfc -> [4096, 1]

Strategy: pure data-parallel over batch (512 per core, 8 cores).
Per core, everything stays resident in SBUF; the T=1024 recurrence is fully
unrolled.  Layout is gate-major: the step matmul produces gates [4H=128
partitions, B=512 free] with weights as the stationary operand.

Per-timestep op schedule (iteration t):
  - DMA x_t row -> state slot (4-slot ring, gives the DMA ~4 steps of slack)
  - MM1: W1^T @ [x_t; h1_{t-1}]            -> G1 PSUM [128,512]
  - sigma1 = sigmoid(G1[ifo] + b1)          (ACT, bias per partition)
  - g1~    = tanh(G1[g] + b1g)
  - c1     = f1*c1 + i1*g1~                 (DVE bf16 2x)
  - th     = tanh(C[0:64])                  computes tanh(c1_t) AND tanh(c2_{t-1})
  - h1_t   = o1*th1  (written twice: rhs rows for MM1(t+1) and MM2(t))
  - h2_{t-1} = o2_{t-1}*th2                 (deferred one step; lands next to h1_t)
  - MM2: W2^T @ [h1_t; h2_{t-1}]            -> G2 PSUM [128,512]
  - sigma2 / g2~ / c2-update                (th2/h2_t deferred to iteration t+1)

Gate order is permuted from PyTorch's [i,f,g,o] to [i,f,o,g] so the three
sigmoid gates are contiguous partitions (one ACT instr) and tanh-gate separate.

The final FC ([4096,32] @ [32,1]) is done on host in numpy.
"""

import numpy as np
import ml_dtypes

BF16 = ml_dtypes.bfloat16

H = 32
T = 1024
B_TOTAL = 4096
N_CORES = 8
B = B_TOTAL // N_CORES  # 512 per core
R = 8  # x-row refill granularity (ring is 2R slots)
KERNEL_K = 1  # independent batch chains per core

_PERM = np.concatenate([
    np.arange(0, 32),      # i
    np.arange(32, 64),     # f
    np.arange(96, 128),    # o
    np.arange(64, 96),     # g
])


def build_bass(Tn=T, Bn=B, xt_rows=None, K=1, R=8, merged_tanhc=False):
    """K independent batch chains of width Bn/K; R-deep state-slot ring.

    All per-chain tiles are free-dim column slices of shared tiles, so the
    instruction structure is identical per chain and chains interleave on the
    engines to hide the per-step dependency-chain latency.

    xT input is chain-major: [K, Tn, Bc] so the once-per-R-steps x DMA for a
    chain reads a contiguous [R, Bc] block.
    """
    import concourse.bass as bass
    import concourse.bacc as bacc
    import concourse.tile as tile
    from concourse import mybir

    f32 = mybir.dt.float32
    bf16 = mybir.dt.bfloat16
    AF = mybir.ActivationFunctionType

    Bc = Bn // K
    assert Tn % R == 0

    nc = bacc.Bacc(None, target_bir_lowering=False)
    xT = nc.declare_dram_parameter("xT", [K, xt_rows or Tn, Bc], bf16, isOutput=False)
    w12 = nc.declare_dram_parameter("w12", [128, 128], bf16, isOutput=False)
    w2x = nc.declare_dram_parameter("w2x", [128, 128], bf16, isOutput=False)
    bias = nc.declare_dram_parameter("bias", [128, 2], f32, isOutput=False)
    out = nc.declare_dram_parameter("h2_last", [32, Bn], bf16, isOutput=True)

    with tile.TileContext(nc) as tc:
        with (
            tc.tile_pool(name="singles", bufs=1) as singles,
            tc.tile_pool(name="psum", bufs=8, space="PSUM") as psum,
        ):
            WS = singles.tile([128, 128], bf16)  # rows 0:33 = [Whh0;Wih0], 64:96 = Whh1
            W2X = singles.tile([128, 128], bf16)  # rows 64:128 = [Wih1; Whh1]
            BIAS = singles.tile([128, 2], f32)
            nc.sync.dma_start(WS[:], w12[:])
            nc.sync.dma_start(W2X[:], w2x[:])
            nc.sync.dma_start(BIAS[:], bias[:])

            # Big state tile; 2R slots per chain (x rows double-buffered in
            # halves of R).  rows: 0:32 h1, 32 x_t, 64:96 h2
            NS = 2 * R
            STB = singles.tile([128, K * NS * Bc], bf16)
            C = singles.tile([64, K * 2 * Bc], bf16)    # rows 32:64; L1/L2 per chain
            TH = singles.tile([96, K * 2 * Bc], bf16)   # rows 64:96
            SIG = singles.tile([96, K * 2 * Bc], bf16)  # [i;f;o]
            GT = singles.tile([32, K * 2 * Bc], bf16)
            TI = singles.tile([64, K * 2 * Bc], bf16)   # rows 32:64
            OUTT = singles.tile([32, Bn], bf16)

            def slot(c, r):
                off = (c * NS + (r % NS)) * Bc
                return STB[:, off:off + Bc]

            def lcol(tile_, c, l):  # per-(chain, layer) column slice
                off = (c * 2 + l) * Bc
                return tile_[:, off:off + Bc]

            for c in range(K):
                nc.vector.memset(slot(c, 0)[0:32, :], 0.0)      # h1_{-1}
                nc.vector.memset(slot(c, 1)[96:128, :], 0.0)    # h2_{-1}
            nc.vector.memset(C[32:64, :], 0.0)

            b1s = BIAS[0:96, 0:1]
            b1g = BIAS[96:128, 0:1]
            b2s = BIAS[0:96, 1:2]
            b2g = BIAS[96:128, 1:2]

            def xdma(c, t0):
                # rows t0..t0+R-1 of chain c -> x rows (p32) of slots t0%NS..+R-1
                s = (c * NS + (t0 % NS)) * Bc
                dst = STB[32:33, s:s + R * Bc]
                nc.sync.dma_start(dst, xT[c, t0:t0 + R, :].rearrange("t b -> (t b)")[None, :])

            for c in range(K):
                xdma(c, 0)

            for t in range(Tn):
                for c in range(K):
                    s0 = slot(c, t)
                    s1 = slot(c, t + 1)
                    sg = lcol(SIG, c, 0)
                    gt = lcol(GT, c, 0)
                    ti = lcol(TI, c, 0)
                    cc = lcol(C, c, 0)
                    th = lcol(TH, c, 0)
                    sg2 = lcol(SIG, c, 1)
                    gt2 = lcol(GT, c, 1)
                    ti2 = lcol(TI, c, 1)
                    cc2 = lcol(C, c, 1)
                    th2 = lcol(TH, c, 1)

                    G1 = psum.tile([128, Bc], f32, tag="G")
                    nc.tensor.matmul(G1[:], WS[0:33, :], s0[0:33, :],
                                     start=True, stop=True)
                    # L1 gate activations for step t
                    nc.scalar.activation(sg, G1[0:96, :], AF.Sigmoid, bias=b1s)
                    nc.scalar.activation(gt, G1[96:128, :], AF.Tanh, bias=b1g)
                    # L2 gate activations for step t-1 (G2 from last iteration)
                    if t > 0:
                        nc.scalar.activation(sg2, G2[0:96, :], AF.Sigmoid, bias=b2s)
                        nc.scalar.activation(gt2, G2[96:128, :], AF.Tanh, bias=b2g)
                    # L1 cell update (t)
                    nc.vector.tensor_mul(ti[32:64, :], sg[0:32, :], gt[0:32, :])
                    nc.vector.tensor_mul(cc[32:64, :], sg[32:64, :], cc[32:64, :])
                    nc.vector.tensor_add(cc[32:64, :], cc[32:64, :], ti[32:64, :])
                    # L2 cell update (t-1)
                    if t > 0:
                        nc.vector.tensor_mul(ti2[32:64, :], sg2[0:32, :], gt2[0:32, :])
                        nc.vector.tensor_mul(cc2[32:64, :], sg2[32:64, :], cc2[32:64, :])
                        nc.vector.tensor_add(cc2[32:64, :], cc2[32:64, :], ti2[32:64, :])
                    nc.scalar.activation(th[64:96, :], cc[32:64, :], AF.Tanh)
                    if t > 0:
                        nc.scalar.activation(th2[64:96, :], cc2[32:64, :], AF.Tanh)
                    # h1_t (both copies), h2_{t-1}
                    nc.vector.tensor_mul(s1[64:96, :], sg[64:96, :], th[64:96, :])
                    nc.vector.tensor_mul(s1[0:32, :], sg[64:96, :], th[64:96, :])
                    if t > 0:
                        nc.vector.tensor_mul(s1[96:128, :], sg2[64:96, :],
                                             th2[64:96, :])

                    G2 = psum.tile([128, Bc], f32, tag="G")
                    nc.tensor.matmul(G2[:], W2X[64:128, :], s1[64:128, :],
                                     start=True, stop=True)

                    # refill x rows for the slot ring, one DMA per R steps
                    if t % R == 0 and t + R < Tn:
                        xdma(c, t + R)

            # epilogue: finish L2 ladder for step Tn-1 and emit h2_last
            for c in range(K):
                sg2 = lcol(SIG, c, 1)
                gt2 = lcol(GT, c, 1)
                ti2 = lcol(TI, c, 1)
                cc2 = lcol(C, c, 1)
                th2 = lcol(TH, c, 1)
                nc.scalar.activation(sg2, G2[0:96, :], AF.Sigmoid, bias=b2s)
                nc.scalar.activation(gt2, G2[96:128, :], AF.Tanh, bias=b2g)
                nc.vector.tensor_mul(ti2[32:64, :], sg2[0:32, :], gt2[0:32, :])
                nc.vector.tensor_mul(cc2[32:64, :], sg2[32:64, :], cc2[32:64, :])
                nc.vector.tensor_add(cc2[32:64, :], cc2[32:64, :], ti2[32:64, :])
                nc.scalar.activation(th2[64:96, :], cc2[32:64, :], AF.Tanh)
                nc.vector.tensor_mul(OUTT[:, c * Bc:(c + 1) * Bc],
                                     sg2[64:96, :], th2[64:96, :])
            nc.sync.dma_start(out[:], OUTT[:])

    if not nc.is_finalized():
        nc.finalize()
    return nc


def _prep_shared(Wih0, Whh0, bih0, bhh0, Wih1, Whh1, bih1, bhh1):
    p = _PERM
    w12 = np.zeros((128, 128), np.float32)
    w12[0:32] = Whh0[p, :].T
    w12[32:33] = Wih0[p, 0:1].T
    w12[64:96] = Whh1[p, :].T
    w2x = np.zeros((128, 128), np.float32)
    w2x[64:96] = Wih1[p, :].T
    w2x[96:128] = Whh1[p, :].T
    bias = np.stack([(bih0 + bhh0)[p], (bih1 + bhh1)[p]], axis=1)  # [128, 2]
    return w12.astype(BF16), w2x.astype(BF16), bias.astype(np.float32)


def kernel(x, Wih0, Whh0, bih0, bhh0, Wih1, Whh1, bih1, bhh1, Wfc, bfc):
    from concourse.bass_utils import run_bass_kernel_spmd

    x = np.asarray(x, np.float32)
    w12, w2x, bias = _prep_shared(
        np.asarray(Wih0, np.float32), np.asarray(Whh0, np.float32),
        np.asarray(bih0, np.float32), np.asarray(bhh0, np.float32),
        np.asarray(Wih1, np.float32), np.asarray(Whh1, np.float32),
        np.asarray(bih1, np.float32), np.asarray(bhh1, np.float32))

    nc = build_bass(T, B, K=KERNEL_K)

    in_maps = []
    K = KERNEL_K
    Bc = B // K
    for c in range(N_CORES):
        xc = x[c * B:(c + 1) * B, :, 0]          # [B, T]
        xTc = np.stack([np.ascontiguousarray(xc[k * Bc:(k + 1) * Bc, :].T)
                        for k in range(K)], axis=0).astype(BF16)  # [K, T, Bc]
        in_maps.append({"xT": xTc, "w12": w12, "w2x": w2x, "bias": bias})

    res = run_bass_kernel_spmd(nc, in_maps, core_ids=list(range(N_CORES)))

    Wfc = np.asarray(Wfc, np.float32)
    bfc = np.asarray(bfc, np.float32)
    outs = []
    for c in range(N_CORES):
        h2 = np.asarray(res.results[c]["h2_last"], dtype=np.float32)  # [32, B]
        outs.append(h2.T @ Wfc.T + bfc)          # [B, 1]
    return np.concatenate(outs, axis=0).astype(np.float32)



# revision 29
# speedup vs baseline: 1.7564x; 1.7564x over previous
"""Trainium2 Bass kernel for 2-layer LSTM (H=32, in=1) + final-step FC.

Problem: x [4096, 1024, 1] -> 2x LSTM(H=32) -> h2[:, -1, :] @ Wfc.T + bfc -> [4096, 1]

Strategy: pure data-parallel over batch (512 per core, 8 cores), with each
core's batch split into K independent chains that pipeline on the engines
to hide the per-step dependency-chain latency.

Per-core, per-chain layout (all SBUF-resident, T=1024 recurrence unrolled):
  - One PSUM bank per chain holds BOTH layers' gate pre-activations
    side by side: cols 0:Bc = G1_t, cols Bc:2Bc = G2_{t-1}.  A single
    sigmoid ACT instruction [128, 2*Bc] activates both layers' gates.
  - Layer biases are folded into the matmuls via a constant-1.0 row in
    the moving operand (lhsT row 33 = bias vector), so the shared sigmoid
    needs no per-layer bias.
  - All four gates use sigmoid; the g-gate weight rows are pre-scaled x2
    so tanh(x) = 2*sigmoid(2x) - 1 is recovered with a tensor_scalar
    fixup plus v = i*g~, c = u + v per layer, split across DVE and
    GPSIMD so every two-input op reads both operands from the same
    physical partitions (hardware constraint).
  - tanh(c1_t) and tanh(c2_{t-1}) are partition-stacked into ONE ACT
    instruction ([64, Bc]; c1 rows 0:32, c2 rows 32:64).
  - MM1 and MM2 share one 96-row slot ring [h1; x; 1; pad; h2] (weight
    tiles zero the rows they ignore), so h1_t and h2_{t-1} are written
    once and each matmul is a single instruction.  MM1 reads only rows
    0:34, so it does not wait on h2.
  - Layer 2 runs one step deferred; all its elementwise work sits on the
    otherwise-idle GPSIMD engine (only ~30% slower than DVE per op).

Gate order is permuted from PyTorch's [i,f,g,o] to [i,f,o,g].
The final FC ([4096,32] @ [32,1]) is done on host in numpy.
"""

import numpy as np
import ml_dtypes

BF16 = ml_dtypes.bfloat16

H = 32
T = 1024
B_TOTAL = 4096
N_CORES = 8
B = B_TOTAL // N_CORES  # 512 per core
KERNEL_K = 2  # independent batch chains per core
KERNEL_R = 8  # x-row refill granularity (ring is 2R slots)

# Per-layer gate orders chosen so every two-input DVE/GPSIMD op reads both
# operands from the SAME physical partitions (hardware requirement; outputs
# may be partition-shifted on DVE/ACT but not on GPSIMD).
# L1 rows: f@0:32, i@32:64, o@64:96, g@96:128
# L2 rows: i@0:32, f@32:64, g@64:96, o@96:128
_PERM1 = np.concatenate([
    np.arange(32, 64),     # f
    np.arange(0, 32),      # i
    np.arange(96, 128),    # o
    np.arange(64, 96),     # g
])
_PERM2 = np.concatenate([
    np.arange(0, 32),      # i
    np.arange(32, 64),     # f
    np.arange(64, 96),     # g
    np.arange(96, 128),    # o
])


def chain_sizes(Bn, K):
    base = Bn // K
    return [base + (1 if i < Bn % K else 0) for i in range(K)]


def build_bass(Tn=T, Bn=B, K=KERNEL_K, R=KERNEL_R, debug=False, u1_dve=False):
    import concourse.bacc as bacc
    import concourse.tile as tile
    from concourse import mybir
    from concourse.alu_op_type import AluOpType

    f32 = mybir.dt.float32
    bf16 = mybir.dt.bfloat16
    AF = mybir.ActivationFunctionType

    Bcs = chain_sizes(Bn, K)
    OFF = np.concatenate([[0], np.cumsum(Bcs)]).tolist()
    assert Tn % R == 0
    NS = 2 * R  # slots per chain ring

    nc = bacc.Bacc(None, target_bir_lowering=False)
    # xT input is a flat chain-major buffer: chain c's [Tn, Bc_c] block
    # (time-major) lives at offset OFF[c]*Tn, so the once-per-R-steps x DMA
    # reads a contiguous [R*Bc_c] run.
    xT = nc.declare_dram_parameter("xT", [1, Tn * Bn], bf16, isOutput=False)
    # Slot rows: 0:32 h1, 32 x, 33 ones, 34:96 zero pad, 96:128 h2.
    # MM1 reads rows 0:34 only (no h2 dependency); MM2 reads rows 0:128.
    # W1 rows: 0:32 Whh0^T, 32 Wih0^T, 33 bias1.
    # W2 rows: 0:32 Wih1^T, 32 zeros, 33 bias2, 34:96 zeros, 96:128 Whh1^T.
    w1 = nc.declare_dram_parameter("w1", [34, 128], bf16, isOutput=False)
    w2 = nc.declare_dram_parameter("w2", [128, 128], bf16, isOutput=False)
    out = nc.declare_dram_parameter("h2_last", [32, Bn], bf16, isOutput=True)
    if debug:
        dbg_ts = nc.declare_dram_parameter("dbg_ts", [128, 2 * Bn], bf16,
                                           isOutput=True)
        dbg_c = nc.declare_dram_parameter("dbg_c", [64, Bn], bf16,
                                          isOutput=True)
        dbg_th = nc.declare_dram_parameter("dbg_th", [128, Bn], bf16,
                                           isOutput=True)
        dbg_slot = nc.declare_dram_parameter("dbg_slot", [128, Bn], bf16,
                                             isOutput=True)
        dbg_gt = nc.declare_dram_parameter("dbg_gt", [64, Bn], bf16,
                                           isOutput=True)
        dbg_u = nc.declare_dram_parameter("dbg_u", [64, Bn], bf16,
                                          isOutput=True)
        dbg_v = nc.declare_dram_parameter("dbg_v", [64, Bn], bf16,
                                          isOutput=True)

    with tile.TileContext(nc) as tc:
        with (
            tc.tile_pool(name="singles", bufs=1) as singles,
            tc.tile_pool(name="psum", bufs=1, space="PSUM") as psum,
        ):
            W1 = singles.tile([34, 128], bf16)
            W2 = singles.tile([128, 128], bf16)
            nc.sync.dma_start(W1[:], w1[:])
            nc.sync.dma_start(W2[:], w2[:])

            # Slot ring shared by MM1 and MM2 (rows as above); NS slots per
            # chain, chain c's slots at columns NS*OFF[c] + r*Bc_c.
            STB = singles.tile([128, NS * Bn], bf16)
            # Sigmoid outputs: per chain [128, 2*Bc] (cols: L1 then L2)
            TS = singles.tile([128, 2 * Bn], bf16)
            # States/scratch.  Partition homes (physical):
            #   c1@0:32, c2@32:64; th1@64:96, th2@96:128;
            #   u1/v1@0:32, u2/v2@32:64; g~1@32:64, g~2@0:32.
            C = singles.tile([64, Bn], bf16)
            TH = singles.tile([128, Bn], bf16)
            U = singles.tile([64, Bn], bf16)
            V = singles.tile([64, Bn], bf16)
            GT = singles.tile([64, Bn], bf16)
            OUTT = singles.tile([32, Bn], bf16)

            # Per-chain [128, 2*Bc] f32 PSUM bank: cols 0:Bc G1, Bc:2Bc G2.
            G = [psum.tile([128, 2 * Bcs[c]], f32, tag=f"G{c}", name=f"G{c}")
                 for c in range(K)]

            def slot(c, r):
                off = NS * OFF[c] + (r % NS) * Bcs[c]
                return STB[:, off:off + Bcs[c]]

            def col(tile_, c):  # per-chain column slice of a [*, Bn] tile
                return tile_[:, OFF[c]:OFF[c] + Bcs[c]]

            def tcol(c, l):  # per-(chain, layer) column slice of TS
                off = 2 * OFF[c] + l * Bcs[c]
                return TS[:, off:off + Bcs[c]]

            for c in range(K):
                nc.vector.memset(slot(c, 0)[0:32, :], 0.0)   # h1_{-1}
                nc.vector.memset(slot(c, 1)[96:128, :], 0.0)  # h2_{-1}
            # zero pad rows 34:96, then bias row 33 = 1.0 (partition starts
            # must be 32-aligned, so write 32:64 then re-write 32:34);
            # row 32 is overwritten by every x DMA
            nc.vector.memset(STB[32:64, :], 0.0)
            nc.vector.memset(STB[64:96, :], 0.0)
            nc.vector.memset(STB[32:34, :], 1.0)
            nc.vector.memset(C[:], 0.0)                      # c1_{-1}, c2_{-1}
            # Safe L2 gate values for the t=0 iteration (i=o=0 -> c2,h2
            # stay 0); the t=0 sigmoid only covers the G1 half.
            nc.vector.memset(TS[:], 0.0)

            def xdma(c, t0):
                # x rows t0..t0+R-1 of chain c -> x rows (p32) of R slots
                s = NS * OFF[c] + (t0 % NS) * Bcs[c]
                dst = STB[32:33, s:s + R * Bcs[c]]
                src_off = OFF[c] * Tn + t0 * Bcs[c]
                nc.sync.dma_start(dst, xT[:, src_off:src_off + R * Bcs[c]])

            for c in range(K):
                xdma(c, 0)

            # MM1 for t=0 (steady-state MM1(t+1) is emitted inside iter t,
            # before MM2(t), so PE's in-order weight loads don't make the
            # h2-waiting MM2 block a ready MM1).
            for c in range(K):
                nc.tensor.matmul(G[c][:, 0:Bcs[c]], W1[:], slot(c, 0)[0:34, :],
                                 start=True, stop=True)
            for t in range(Tn):
                for c in range(K):
                    # Joint sigmoid over G1_t | G2_{t-1} (G2 half only valid
                    # from t>=1; at t=0 cover just G1).
                    if t > 0:
                        nc.scalar.activation(
                            TS[:, 2 * OFF[c]:2 * OFF[c] + 2 * Bcs[c]],
                            G[c][:], AF.Sigmoid)
                    else:
                        nc.scalar.activation(tcol(c, 0), G[c][:, 0:Bcs[c]],
                                             AF.Sigmoid)
                for c in range(K):
                    T1, T2 = tcol(c, 0), tcol(c, 1)
                    cc, uu, vv, gt = col(C, c), col(U, c), col(V, c), col(GT, c)
                    # ---- L1 cell update (step t): u1 on GPSIMD, rest DVE ----
                    if u1_dve:
                        nc.vector.tensor_mul(uu[0:32, :], T1[0:32, :],
                                             cc[0:32, :])
                    else:
                        nc.gpsimd.tensor_mul(uu[0:32, :], T1[0:32, :],
                                             cc[0:32, :])
                    nc.vector.tensor_scalar(gt[32:64, :], T1[96:128, :], 2.0,
                                            -1.0, AluOpType.mult,
                                            AluOpType.add)
                    nc.vector.tensor_mul(vv[0:32, :], T1[32:64, :],
                                         gt[32:64, :])
                    nc.vector.tensor_add(cc[0:32, :], uu[0:32, :], vv[0:32, :])
                    # ---- L2 cell update (step t-1): gfix/v on DVE, rest
                    # GPSIMD (all-same-partition ops only) ----
                    nc.gpsimd.tensor_mul(uu[32:64, :], T2[32:64, :],
                                         cc[32:64, :])
                    nc.vector.tensor_scalar(gt[0:32, :], T2[64:96, :], 2.0,
                                            -1.0, AluOpType.mult,
                                            AluOpType.add)
                    nc.vector.tensor_mul(vv[32:64, :], T2[0:32, :],
                                         gt[0:32, :])
                    nc.gpsimd.tensor_add(cc[32:64, :], uu[32:64, :],
                                         vv[32:64, :])
                for c in range(K):
                    # tanh over [c1_t ; c2_{t-1}] stacked; out shifted +64 so
                    # th1 lands at o1's partitions (64:96), th2 at o2's
                    # (96:128)
                    nc.scalar.activation(col(TH, c)[64:128, :],
                                         col(C, c)[0:64, :], AF.Tanh)
                for c in range(K):
                    # h1_t -> slot rows (feeds MM1_{t+1} and MM2_t)
                    nc.vector.tensor_mul(slot(c, t + 1)[0:32, :],
                                         tcol(c, 0)[64:96, :],
                                         col(TH, c)[64:96, :])
                    # h2_{t-1} -> slot rows 96:128 (GPSIMD, all @96:128)
                    nc.gpsimd.tensor_mul(slot(c, t + 1)[96:128, :],
                                         tcol(c, 1)[96:128, :],
                                         col(TH, c)[96:128, :])
                for c in range(K):
                    # MM1 for step t+1 (needs only h1_t; emitted before MM2
                    # so it isn't stuck behind MM2's wait for h2 on PE)
                    if t + 1 < Tn:
                        nc.tensor.matmul(G[c][:, 0:Bcs[c]], W1[:],
                                         slot(c, t + 1)[0:34, :],
                                         start=True, stop=True)
                    # MM2: G2 cols = [Wih1;0;b2;0;Whh1] @ slot rows 0:96
                    nc.tensor.matmul(G[c][:, Bcs[c]:2 * Bcs[c]], W2[:],
                                     slot(c, t + 1)[:], start=True, stop=True)
                if t % R == 0 and t + R < Tn:
                    for c in range(K):
                        xdma(c, t + R)

            # epilogue: finish L2 step Tn-1 and emit h2_last (all on DVE)
            for c in range(K):
                T2 = tcol(c, 1)
                cc = col(C, c)
                th = col(TH, c)
                uu = col(U, c)
                vv = col(V, c)
                gt = col(GT, c)
                nc.scalar.activation(T2, G[c][:, Bcs[c]:2 * Bcs[c]], AF.Sigmoid)
                nc.vector.tensor_mul(uu[32:64, :], T2[32:64, :], cc[32:64, :])
                nc.vector.tensor_scalar(gt[0:32, :], T2[64:96, :], 2.0, -1.0,
                                        AluOpType.mult, AluOpType.add)
                nc.vector.tensor_mul(vv[32:64, :], T2[0:32, :], gt[0:32, :])
                nc.vector.tensor_add(cc[32:64, :], uu[32:64, :], vv[32:64, :])
                # tanh in place (32:64), bring o2 down with a copy so the
                # h2 multiply's inputs share partitions
                nc.scalar.activation(th[32:64, :], cc[32:64, :], AF.Tanh)
                nc.vector.tensor_copy(gt[32:64, :], T2[96:128, :])
                nc.vector.tensor_mul(OUTT[:, OFF[c]:OFF[c] + Bcs[c]],
                                     gt[32:64, :], th[32:64, :])
            nc.sync.dma_start(out[:], OUTT[:])
            if debug:
                nc.sync.dma_start(dbg_ts[:], TS[:])
                nc.sync.dma_start(dbg_c[:], C[:])
                nc.sync.dma_start(dbg_th[64:128, :], TH[64:128, :])
                nc.sync.dma_start(dbg_gt[:], GT[:])
                nc.sync.dma_start(dbg_u[:], U[:])
                nc.sync.dma_start(dbg_v[:], V[:])
                nc.sync.dma_start(dbg_slot[:, 0:Bcs[0]],
                                  STB[:, Bcs[0]:2 * Bcs[0]])

    if not nc.is_finalized():
        nc.finalize()
    return nc


def _prep_shared(Wih0, Whh0, bih0, bhh0, Wih1, Whh1, bih1, bhh1):
    """Build the lhsT tiles W1 [34,128], W2 [96,128].  Gate rows permuted
    to [i,f,o,g]; g-gate rows/biases scaled x2 (tanh-via-sigmoid trick).
    Moving-operand rows: 0:32 h1, 32 x, 33 const-1, 34:64 pad, 64:96 h2."""
    p1, p2 = _PERM1, _PERM2
    scale1 = np.ones((128, 1), np.float32)
    scale1[96:128] = 2.0  # L1 g rows
    scale = np.ones((128, 1), np.float32)
    scale[64:96] = 2.0    # L2 g rows
    W1m = np.zeros((34, 128), np.float32)
    W1m[0:32] = (Whh0[p1, :] * scale1).T
    W1m[32:33] = (Wih0[p1, 0:1] * scale1).T
    W1m[33] = (bih0 + bhh0)[p1] * scale1[:, 0]
    W2m = np.zeros((128, 128), np.float32)
    W2m[0:32] = (Wih1[p2, :] * scale).T
    W2m[33] = (bih1 + bhh1)[p2] * scale[:, 0]
    W2m[96:128] = (Whh1[p2, :] * scale).T
    return W1m.astype(BF16), W2m.astype(BF16)


def _prep_x_core(xc, Tn=T, K=KERNEL_K):
    """xc [B, Tn] f32 for one core -> flat chain-major xT [1, Tn*B] bf16."""
    Bcs = chain_sizes(xc.shape[0], K)
    off = 0
    parts = []
    for k in range(K):
        parts.append(np.ascontiguousarray(xc[off:off + Bcs[k], :].T).ravel())
        off += Bcs[k]
    return np.concatenate(parts)[None, :].astype(BF16)


def _prep_inputs(x):
    """x [B_TOTAL, T, 1] f32 -> per-core flat chain-major xT arrays."""
    return [_prep_x_core(x[c * B:(c + 1) * B, :, 0], T, KERNEL_K)
            for c in range(N_CORES)]


def kernel(x, Wih0, Whh0, bih0, bhh0, Wih1, Whh1, bih1, bhh1, Wfc, bfc):
    from concourse.bass_utils import run_bass_kernel_spmd

    x = np.asarray(x, np.float32)
    w1, w2 = _prep_shared(
        np.asarray(Wih0, np.float32), np.asarray(Whh0, np.float32),
        np.asarray(bih0, np.float32), np.asarray(bhh0, np.float32),
        np.asarray(Wih1, np.float32), np.asarray(Whh1, np.float32),
        np.asarray(bih1, np.float32), np.asarray(bhh1, np.float32))

    nc = build_bass(T, B, K=KERNEL_K, R=KERNEL_R)

    in_maps = [{"xT": xTc, "w1": w1, "w2": w2}
               for xTc in _prep_inputs(x)]

    res = run_bass_kernel_spmd(nc, in_maps, core_ids=list(range(N_CORES)))

    Wfc = np.asarray(Wfc, np.float32)
    bfc = np.asarray(bfc, np.float32)
    outs = []
    for c in range(N_CORES):
        h2 = np.asarray(res.results[c]["h2_last"], dtype=np.float32)  # [32, B]
        outs.append(h2.T @ Wfc.T + bfc)          # [B, 1]
    return np.concatenate(outs, axis=0).astype(np.float32)
